# revision 1
# baseline (speedup 1.0000x reference)
"""Trainium2 Bass kernel for nn_AdaptiveLinearWithChannel.

out[b,c,n,:] = x[b,c,n,:] @ weight[indices[c]] + bias[c,0,:] + hyper(t[b], c)
with hyper = per-channel relu MLP (1 -> 64 -> 64 -> 32) / DIN.

Sharding: channel dim split across 8 NeuronCores (16 channels each,
expert-parallel). Per-channel weight/bias/hyper gathers (64KB) happen on host
as part of sharding; all FLOPs over x (the 512MB tensor) and the hyper MLP
run on device.

Device dataflow (v2, feature-on-partition layout):
  - host packs xT[b, g, 4ch*32feat, n] bf16 (transpose + downcast on host)
  - per (b, g): one contiguous 2MB DMA per 8192-point slab into SBUF
  - matmul: stationary 128x128 block-diagonal weight (4 channels), moving
    operand = xT slab columns; writes bf16 PSUM (packed)
  - per-(b,channel,feat) shift = bias + hyper(t)/DIN is a per-PARTITION
    constant in this layout: fused into the PSUM->SBUF eviction as a
    bias-add (split between ACT scalar.add and DVE tensor_scalar_add)
  - one contiguous 2MB DMA per slab back to HBM in [co, n] layout;
    host un-transposes + upcasts to f32
The tiny hyper MLP runs on device at kernel start (block-diag pairs of
channels, PE transposes via id2), producing the per-partition shift vector.
"""

import sys

for _p in ("/opt/trn_rl_repo", "/opt/pypackages"):
    if _p not in sys.path:
        sys.path.append(_p)

import numpy as np
import ml_dtypes

import concourse.bass as bass
import concourse.mybir as mybir
from concourse import bacc
import concourse.tile as tile

B, C, N, DIN, DOUT, HID = 2, 128, 16384, 32, 32, 64
NCORES = 8
CS = C // NCORES          # channels per core = 16
G = CS // 4               # channel groups of 4 (partition block = 4*32 = 128)
NPAIR = CS // 2           # hyper block-diag pairs = 8

F32 = mybir.dt.float32
BF16 = mybir.dt.bfloat16
BF16_NP = ml_dtypes.bfloat16


def build_nc(n_points=N, reps=1, xt_copy_engine=None, slab_pts=16384,
             evict_pattern=("act",), mm_cols=512,
             xs_bufs=2, os_bufs=3, mmp_bufs=3, out_dma="gpsimd",
             in_dma="sync", mode="full", slab_contig=False):
    """Build the per-core Bass graph. Same SPMD graph for all 8 cores.

    n_points: points per channel (16384 production; smaller for simulation).
    reps: repeat whole body in a hardware loop (timing harness only).
    evict_pattern: engine per 1024-col eviction chunk, cycled.
    """
    import json as _json
    import os as _os
    _ov = _json.loads(_os.environ.get("KCFG", "{}"))
    slab_pts = _ov.get("slab_pts", slab_pts)
    evict_pattern = tuple(_ov.get("evict_pattern", evict_pattern))
    mm_cols = _ov.get("mm_cols", mm_cols)
    xs_bufs = _ov.get("xs_bufs", xs_bufs)
    os_bufs = _ov.get("os_bufs", os_bufs)
    mmp_bufs = _ov.get("mmp_bufs", mmp_bufs)
    out_dma = _ov.get("out_dma", out_dma)
    in_dma = _ov.get("in_dma", in_dma)
    reorder = _ov.get("reorder", True)
    slab_pts = min(slab_pts, n_points)
    assert n_points % slab_pts == 0
    n_slabs = n_points // slab_pts
    EV = 1024                 # eviction chunk columns (2 PSUM banks, f32)
    assert slab_pts % EV == 0

    nc = bacc.Bacc("TRN2", target_bir_lowering=False, debug=False)

    if slab_contig:
        xT_d = nc.dram_tensor("xT", [B, G, n_slabs, 128, slab_pts], BF16,
                              kind="ExternalInput")
        out_d = nc.dram_tensor("out", [B, G, n_slabs, 128, slab_pts], BF16,
                               kind="ExternalOutput")
        xT_src = lambda b, g, s, n0: xT_d[b, g, s]
        out_dst = lambda b, g, s, n0: out_d[b, g, s]
    else:
        xT_d = nc.dram_tensor("xT", [B, G, 128, n_points], BF16,
                              kind="ExternalInput")
        out_d = nc.dram_tensor("out", [B, G, 128, n_points], BF16,
                               kind="ExternalOutput")
        xT_src = lambda b, g, s, n0: xT_d[b, g, :, n0:n0 + slab_pts]
        out_dst = lambda b, g, s, n0: out_d[b, g, :, n0:n0 + slab_pts]
    tT = nc.dram_tensor("tT", [1, B], F32, kind="ExternalInput")
    wl = nc.dram_tensor("wl", [G, 128, 128], BF16, kind="ExternalInput")
    wh1 = nc.dram_tensor("wh1", [1, CS * HID], F32, kind="ExternalInput")
    b1 = nc.dram_tensor("b1", [B, CS * HID], F32, kind="ExternalInput")
    wh2 = nc.dram_tensor("wh2", [NPAIR, 128, 128], F32, kind="ExternalInput")
    b2 = nc.dram_tensor("b2", [B, CS * HID], F32, kind="ExternalInput")
    wh3 = nc.dram_tensor("wh3", [NPAIR, 128, 2 * DOUT], F32,
                         kind="ExternalInput")
    shiftc = nc.dram_tensor("shiftc", [B, CS * DOUT], F32, kind="ExternalInput")
    id2 = nc.dram_tensor("id2", [B, B], F32, kind="ExternalInput")

    with tile.TileContext(nc) as tc:

        def body():
            with (
                tc.tile_pool(name="const", bufs=1) as const,
                tc.tile_pool(name="xs", bufs=xs_bufs) as xpool,
                tc.tile_pool(name="os", bufs=os_bufs) as opool,
                tc.tile_pool(name="mmp", bufs=mmp_bufs, space="PSUM") as mmpool,
                tc.tile_pool(name="hyp", bufs=1, space="PSUM") as hyppool,
            ):
                # ---- load constants ----
                tT_t = const.tile([1, B], F32)
                nc.sync.dma_start(tT_t[:], tT[:])
                id2_t = const.tile([B, B], F32)
                nc.sync.dma_start(id2_t[:], id2[:])
                wh1_t = const.tile([1, CS * HID], F32)
                nc.sync.dma_start(wh1_t[:], wh1[:])
                b1_t = const.tile([B, CS * HID], F32)
                nc.sync.dma_start(b1_t[:], b1[:])
                b2_t = const.tile([B, CS * HID], F32)
                nc.sync.dma_start(b2_t[:], b2[:])
                shiftc_t = const.tile([B, CS * DOUT], F32)
                nc.sync.dma_start(shiftc_t[:], shiftc[:])
                wl_t = []
                for g in range(G):
                    w = const.tile([128, 128], BF16, tag=f"wl{g}")
                    nc.sync.dma_start(w[:], wl[g])
                    wl_t.append(w)
                wh2_t = []
                wh3_t = []
                for j in range(NPAIR):
                    w = const.tile([128, 128], F32, tag=f"wh2_{j}")
                    nc.sync.dma_start(w[:], wh2[j])
                    wh2_t.append(w)
                    w = const.tile([128, 2 * DOUT], F32, tag=f"wh3_{j}")
                    nc.sync.dma_start(w[:], wh3[j])
                    wh3_t.append(w)

                # ---- hyper MLP (f32, tiny): h = relu(t @ W1 + b1) ... ----
                h1_ps = hyppool.tile([B, CS * HID], F32, tag="hyp")
                nc.tensor.matmul(h1_ps[:, 0:512], tT_t[:], wh1_t[:, 0:512],
                                 start=True, stop=True)
                nc.tensor.matmul(h1_ps[:, 512:1024], tT_t[:], wh1_t[:, 512:1024],
                                 start=True, stop=True)
                h1_sb = const.tile([B, CS * HID], F32)
                nc.vector.tensor_add(h1_sb[:], h1_ps[:], b1_t[:])
                nc.vector.tensor_scalar_max(h1_sb[:], h1_sb[:], 0.0)

                h1T_sb = const.tile([128, 2 * NPAIR], F32)
                for j in range(NPAIR):
                    tp = hyppool.tile([128, B], F32, tag="hyp")
                    nc.tensor.transpose(tp[:], h1_sb[:, j * 128:(j + 1) * 128],
                                        id2_t[:])
                    nc.scalar.copy(h1T_sb[:, j * B:(j + 1) * B], tp[:])

                h2_ps = hyppool.tile([B, CS * HID], F32, tag="hyp")
                for j in range(NPAIR):
                    nc.tensor.matmul(h2_ps[:, j * 128:(j + 1) * 128],
                                     h1T_sb[:, j * B:(j + 1) * B], wh2_t[j][:],
                                     start=True, stop=True)
                h2_sb = const.tile([B, CS * HID], F32)
                nc.vector.tensor_add(h2_sb[:], h2_ps[:], b2_t[:])
                nc.vector.tensor_scalar_max(h2_sb[:], h2_sb[:], 0.0)

                h2T_sb = const.tile([128, 2 * NPAIR], F32)
                for j in range(NPAIR):
                    tp = hyppool.tile([128, B], F32, tag="hyp")
                    nc.tensor.transpose(tp[:], h2_sb[:, j * 128:(j + 1) * 128],
                                        id2_t[:])
                    nc.scalar.copy(h2T_sb[:, j * B:(j + 1) * B], tp[:])

                h3_ps = hyppool.tile([B, CS * DOUT], F32, tag="hyp")
                for j in range(NPAIR):
                    nc.tensor.matmul(h3_ps[:, j * 2 * DOUT:(j + 1) * 2 * DOUT],
                                     h2T_sb[:, j * B:(j + 1) * B], wh3_t[j][:],
                                     start=True, stop=True)
                # shift[b, c*32+o] = h3/DIN + (bias + hb3/DIN)
                shift_sb = const.tile([B, CS * DOUT], F32)
                nc.vector.scalar_tensor_tensor(
                    shift_sb[:], h3_ps[:], 1.0 / DIN, shiftc_t[:],
                    op0=mybir.AluOpType.mult, op1=mybir.AluOpType.add)

                # transpose to per-partition layout: shiftT[(c,o), g*B+b]
                shiftT = const.tile([128, G * B], F32)
                for g in range(G):
                    tp = hyppool.tile([128, B], F32, tag="hyp")
                    nc.tensor.transpose(tp[:],
                                        shift_sb[:, g * 128:(g + 1) * 128],
                                        id2_t[:])
                    nc.scalar.copy(shiftT[:, g * B:(g + 1) * B], tp[:])

                # free-dim broadcast of shiftT columns (for DVE tensor_tensor
                # evictions: nc.vector TS-with-AP-scalar is pathologically
                # slow on HW, but plain TT from PSUM runs at full rate)
                shift_bc = {}
                if "dvett" in evict_pattern:
                    zz = const.tile([128, EV], BF16)
                    nc.vector.memset(zz[:], 0.0)
                    for b in range(B):
                        for g in range(G):
                            sb = const.tile([128, EV], BF16, tag=f"sbc{b}_{g}")
                            nc.scalar.add(sb[:], zz[:],
                                          shiftT[:, g * B + b:g * B + b + 1])
                            shift_bc[(b, g)] = sb

                # ---- main loop (software-pipelined: the in-DMA for slab
                # k+1 is issued BEFORE the out-DMA for slab k, so the SP
                # ring's FIFO order never parks a prefetch behind an
                # eviction-gated store) ----
                in_eng = getattr(nc, in_dma)
                out_eng = getattr(nc, out_dma)
                slabs = [(b, g, s, s * slab_pts)
                         for b in range(B) for g in range(G)
                         for s in range(n_slabs)]
                xs_t = {}

                def issue_in(k):
                    if k < len(slabs):
                        bb, gg, ss, nn0 = slabs[k]
                        xs = xpool.tile([128, slab_pts], BF16)
                        in_eng.dma_start(xs[:], xT_src(bb, gg, ss, nn0))
                        xs_t[k] = xs

                issue_in(0)
                for k, (b, g, s, n0) in enumerate(slabs):
                    if reorder:
                        issue_in(k + 1)
                    xs = xs_t.pop(k)
                    bias_ap = shiftT[:, g * B + b:g * B + b + 1]
                    if mode == "dma_only":
                        out_eng.dma_start(out_dst(b, g, s, n0), xs[:])
                        continue
                    os_ = opool.tile([128, slab_pts], BF16)
                    for e in range(slab_pts // EV):
                        mm = mmpool.tile([128, EV], F32)
                        for j in range(EV // mm_cols):
                            o0 = j * mm_cols
                            nc.tensor.matmul(
                                mm[:, o0:o0 + mm_cols], wl_t[g][:],
                                xs[:, e * EV + o0:e * EV + o0 + mm_cols],
                                start=True, stop=True)
                        eng = evict_pattern[e % len(evict_pattern)]
                        dst = os_[:, e * EV:(e + 1) * EV]
                        if eng == "act":
                            nc.scalar.add(dst, mm[:], bias_ap)
                        elif eng == "dvett":
                            nc.vector.tensor_add(dst, mm[:],
                                                 shift_bc[(b, g)][:])
                        else:
                            nc.vector.tensor_scalar_add(dst, mm[:], bias_ap)
                    if mode == "full" or (b, g, s) == (0, 0, 0):
                        out_eng.dma_start(out_dst(b, g, s, n0), os_[:])
                    if not reorder:
                        issue_in(k + 1)

        if reps == 1:
            body()
        else:
            with tc.For_i(0, reps, 1):
                body()

    nc.compile()
    return nc


def host_pack(x, indices, t, weight, bias, hW1, hb1, hW2, hb2, hW3, hb3,
              n_points=N, slab_contig=False, slab_pts=8192):
    """Gather per-core channel shards + pack device input tensors."""
    idx = np.asarray(indices).astype(np.int64)
    x = np.asarray(x, dtype=np.float32)
    in_maps = []
    for m in range(NCORES):
        c0 = m * CS
        ci = idx[c0:c0 + CS]
        wg = np.asarray(weight, np.float32)[ci]            # (CS,32,32)
        # NOTE: reference adds bias positionally (no indices gather)
        biasg = np.asarray(bias, np.float32)[c0:c0 + CS, 0, :]  # (CS,32)
        h1w = np.asarray(hW1, np.float32)[ci][:, 0, :]     # (CS,64)
        h1b = np.asarray(hb1, np.float32)[ci]              # (CS,64)
        h2w = np.asarray(hW2, np.float32)[ci]              # (CS,64,64)
        h2b = np.asarray(hb2, np.float32)[ci]              # (CS,64)
        h3w = np.asarray(hW3, np.float32)[ci]              # (CS,64,32)
        h3b = np.asarray(hb3, np.float32)[ci]              # (CS,32)

        # block-diagonal stationary weight per 4-channel group:
        # wl[g, ci*32+i, ci*32+o] = w[4g+ci][i, o]
        wlk = np.zeros((G, 128, 128), np.float32)
        for g in range(G):
            for c in range(4):
                wlk[g, 32 * c:32 * c + 32, 32 * c:32 * c + 32] = wg[4 * g + c]
        wh2 = np.zeros((NPAIR, 128, 128), np.float32)
        wh3 = np.zeros((NPAIR, 128, 2 * DOUT), np.float32)
        for j in range(NPAIR):
            wh2[j, 0:64, 0:64] = h2w[2 * j]
            wh2[j, 64:128, 64:128] = h2w[2 * j + 1]
            wh3[j, 0:64, 0:DOUT] = h3w[2 * j]
            wh3[j, 64:128, DOUT:2 * DOUT] = h3w[2 * j + 1]
        shiftc = (biasg + h3b / DIN).reshape(1, -1).repeat(B, 0)

        # xT[b, g, 4c+..., n]: feature-on-partition, bf16, contiguous rows
        xs = x[:, c0:c0 + CS, :n_points, :].astype(BF16_NP)
        xs = np.ascontiguousarray(
            xs.reshape(B, G, 4, n_points, DIN).transpose(0, 1, 2, 4, 3)
        ).reshape(B, G, 128, n_points)
        if slab_contig:
            sp = min(slab_pts, n_points)
            xs = np.ascontiguousarray(
                xs.reshape(B, G, 128, n_points // sp, sp).swapaxes(2, 3))

        in_maps.append({
            "xT": xs,
            "tT": np.ascontiguousarray(np.asarray(t, np.float32).T),
            "wl": wlk.astype(BF16_NP),
            "wh1": h1w.reshape(1, -1).astype(np.float32),
            "b1": h1b.reshape(1, -1).repeat(B, 0).astype(np.float32),
            "wh2": wh2,
            "b2": h2b.reshape(1, -1).repeat(B, 0).astype(np.float32),
            "wh3": wh3,
            "shiftc": shiftc.astype(np.float32),
            "id2": np.eye(B, dtype=np.float32),
        })
    return in_maps


_NC_CACHE = {}


def _get_nc(n_points=N, reps=1):
    key = (n_points, reps)
    if key not in _NC_CACHE:
        _NC_CACHE[key] = build_nc(n_points, reps)
    return _NC_CACHE[key]


def kernel(**inputs):
    import time
    from concourse.bass_utils import run_bass_kernel_spmd
    nc = _get_nc()
    in_maps = host_pack(**inputs)
    last_err = None
    for attempt in range(3):
        try:
            res = run_bass_kernel_spmd(nc, in_maps,
                                       core_ids=list(range(NCORES)))
            outs = []
            for m in range(NCORES):
                o = np.asarray(res.results[m]["out"])   # (B,G,128,N) bf16
                o = o.reshape(B, G, 4, DOUT, N).transpose(0, 1, 2, 4, 3)
                outs.append(o.reshape(B, CS, N, DOUT).astype(np.float32))
            return np.concatenate(outs, axis=1)
        except Exception as e:  # transient NRT_EXEC_UNIT_UNRECOVERABLE etc.
            last_err = e
            time.sleep(20)
    raise last_err


if __name__ == "__main__":
    nc = build_nc()
    n = sum(len(bb.instructions) for bb in nc.main_func.blocks)
    print(f"built ok: {n} instructions")

